# revision 27
# baseline (speedup 1.0000x reference)
"""Trainium2 Bass kernel for ActorMoE (8 experts, dims 512->1024->512->256->64).

Strategy: data-parallel across 8 NeuronCores (2048 rows each), weights
replicated. On-device compute is feature-major (features on partitions,
batch on the free dim) so the stacked expert weights W_l[e] (shape
[in, out]) are directly the matmul lhsT and no transposes are needed.

ELU trick: h' = elu(z)+1 = min(exp(z+b), relu(z+b)+1), computed as
  e = Exp(z + b)                       (ScalarE, bias fused)
  h' = min(e, max(z + (b+1), 1))       (one custom DVE op: ELU_P1_MOE)
The +1 shift is corrected by subtracting colsum(W_next) from the next
layer's bias on the host, so the math is exact.

Matmuls are emitted with same-weight run length 4 (both 2-bank PSUM groups
of an m-tile accumulate together) — consecutive same-weight matmuls are
substantially cheaper on the PE than alternating ones. Expert layers are
software-pipelined (tick e: load(e+1), L0(e), L2(e-1), L1(e), L3(e-1)) so
each layer-boundary ELU drain is covered by other matmul work; the gate
fills the first boundary.

Softmax gate: logits are small (|logit| < ~2) so exp without max-shift is
safe. Weighted sum over experts happens on the final 64-dim outputs with
per-expert gate rows replicated across partitions via broadcast DMA
(bounced through DRAM, since partition-broadcast needs a DRAM source).
"""

import sys

sys.path.insert(0, "/opt/trn_rl_repo")

import numpy as np
import ml_dtypes

BF = ml_dtypes.bfloat16

B, OBS, ACT, E = 16384, 512, 64, 8
DIMS = [512, 1024, 512, 256, 64]
GH = 256
NCORES = 8
BSH = B // NCORES  # 2048
P = 128
FD = 512  # matmul free dim (one PSUM bank of f32)
NT = BSH // FD  # 4 n-tiles per core
NB = 2  # PSUM banks per group (ELU op width = NB*FD)
NG = NT // NB  # groups per m-tile

_cache = {}
_STRIP = {}  # timing experiments only: {"elu": True} or {"act": True}
_PIPE = {"on": True}  # software-pipelined expert/layer emission order
_EBUF_EXTRA = {}  # timing experiments: extra et buffers


def _get_elu_op():
    """Custom DVE op: out = min(in1, max(in0 + s0, 1)).
    With in0 = z (PSUM), s0 = b+1 per-partition, in1 = exp(z+b) from ACT,
    this computes elu(z+b)+1 in a single DVE pass."""
    if "elu_op" in _cache:
        return _cache["elu_op"]
    from concourse.dve_ops import DveOp, OPS
    from concourse.dve_spec import Spec, Src0, Src1, C0, One, maxx, minn, lower
    from concourse.dve_uop import DveOpSpec

    spec = Spec(
        body=minn(Src1, maxx(Src0 + C0, One)),
        reference=lambda in0, in1, s0: np.minimum(
            in1, np.maximum(in0 + s0, 1.0)
        ),
    )
    shas = {}
    for ver in ("v3", "v4"):
        s = DveOpSpec(name="ELU_P1_MOE", opcode=0, uops=lower(spec, ver=ver), rd1_en=True)
        shas[ver] = s.sha(ver)
    op = DveOp("ELU_P1_MOE", spec, subdim=False, uops_sha=shas)
    OPS.append(op)
    # import-time lookup tables don't see post-import appends — patch them
    import concourse.dve_ops as dve_ops_mod

    dve_ops_mod.CUSTOM_DVE_SPECS[op.name] = op.spec
    dve_ops_mod._SUB_OPCODE_FOR_NAME[op.name] = (
        dve_ops_mod._CUSTOM_DVE_ROW_BASE + len(OPS) - 1
    )
    _cache["elu_op"] = op
    return op


def _build(reps=1, nb=NB):
    """Build the Bass graph. reps>1 wraps the whole body in a For_i loop
    (the body is idempotent) — used only for timing via wall-time slope.
    nb = PSUM banks per matmul group (ELU op width nb*FD)."""
    import concourse.bass as bass  # noqa: F401
    from concourse import bacc, mybir
    import concourse.tile as tile

    NB = nb
    NG = NT // NB
    PS_BUFS = 8 // NB
    E_BUFS = PS_BUFS + 2 + _EBUF_EXTRA.get('n', 0)

    f32 = mybir.dt.float32
    bf16 = mybir.dt.bfloat16
    AF = mybir.ActivationFunctionType
    Alu = mybir.AluOpType

    nc = bacc.Bacc(None, target_bir_lowering=False)

    xTd = nc.dram_tensor("xT", [OBS, BSH], bf16, kind="ExternalInput")
    Wd = [
        nc.dram_tensor(f"W{l}", [E, DIMS[l], DIMS[l + 1]], bf16, kind="ExternalInput")
        for l in range(4)
    ]
    # packed biases: [E, 128, MT] with b[e, p, mo] = bias[e, mo*128 + p]
    MTS = [DIMS[l + 1] // P for l in range(3)]  # [8, 4, 2]
    Bd = [
        nc.dram_tensor(f"B{l}", [E, P, MTS[l]], f32, kind="ExternalInput")
        for l in range(3)
    ]
    Bp1d = [
        nc.dram_tensor(f"B{l}p1", [E, P, MTS[l]], f32, kind="ExternalInput")
        for l in range(3)
    ]
    B3d = nc.dram_tensor("B3", [E, ACT, 1], f32, kind="ExternalInput")
    gW0d = nc.dram_tensor("gW0", [OBS, GH], bf16, kind="ExternalInput")
    gW1d = nc.dram_tensor("gW1", [GH, E], bf16, kind="ExternalInput")
    gB0d = nc.dram_tensor("gB0", [P, GH // P], f32, kind="ExternalInput")
    gB0p1d = nc.dram_tensor("gB0p1", [P, GH // P], f32, kind="ExternalInput")
    gB1d = nc.dram_tensor("gB1", [E, 1], f32, kind="ExternalInput")
    outd = nc.dram_tensor("out", [ACT, BSH], f32, kind="ExternalOutput")

    with tile.TileContext(nc) as tc:
        with (
            tc.tile_pool(name="const", bufs=1) as cpool,
            tc.tile_pool(name="wpool", bufs=2) as wpool,
            tc.tile_pool(name="wpool3", bufs=3) as wpool3,
            tc.tile_pool(name="bpool", bufs=3) as bpool,
            tc.tile_pool(name="hpool", bufs=1) as hpool,
            tc.tile_pool(name="epool", bufs=E_BUFS) as epool,
            tc.tile_pool(name="tpool", bufs=2) as tpool,
            tc.tile_pool(name="psum", bufs=PS_BUFS, space="PSUM") as pspool,
            tc.tile_pool(name="dram", bufs=1, space="DRAM") as dpool,
        ):

            def body():
                # ---- load x and gate params ----
                gw0 = cpool.tile([P, OBS // P, GH], bf16, tag="gw0", name="gw0")
                nc.sync.dma_start(gw0[:], gW0d[:].rearrange("(ko p) o -> p ko o", p=P))
                xt = cpool.tile([P, OBS // P, BSH], bf16, tag="xt", name="xt")
                xt_src = xTd[:].rearrange("(ko p) n -> p ko n", p=P)
                for ko in range(OBS // P):
                    nc.sync.dma_start(xt[:, ko : ko + 1, :], xt_src[:, ko : ko + 1, :])
                gw1 = cpool.tile([P, GH // P, E], bf16, tag="gw1", name="gw1")
                nc.sync.dma_start(gw1[:], gW1d[:].rearrange("(ko p) o -> p ko o", p=P))
                gb0t = cpool.tile([P, GH // P], f32, tag="gb0", name="gb0")
                nc.scalar.dma_start(gb0t[:], gB0d[:])
                gb0p1t = cpool.tile([P, GH // P], f32, tag="gb0p1", name="gb0p1")
                nc.scalar.dma_start(gb0p1t[:], gB0p1d[:])
                gb1t = cpool.tile([E, 1], f32, tag="gb1", name="gb1")
                nc.scalar.dma_start(gb1t[:], gB1d[:])

                elu_op = _get_elu_op()

                def elu_wide(ps_flat, bias_ap, biasp1_ap, out_ap, mp=P):
                    if _STRIP.get("elu"):
                        nc.gpsimd.memset(out_ap[:, 0:8], 0.0)
                        return
                    # ps_flat: [mp, NB*FD] PSUM view; one wide ACT + one wide DVE
                    et = epool.tile([P, NB * FD], bf16, tag="e", name="e")[:mp]
                    if _STRIP.get("act"):
                        nc.gpsimd.memset(et[:, 0:8], 0.0)
                    else:
                        nc.scalar.activation(et, ps_flat, AF.Exp, bias=bias_ap)
                    # fused custom DVE: out = min(et, max(z + (b+1), 1)) = elu+1
                    nc.vector._custom_dve(
                        elu_op, out=out_ap, in0=ps_flat, in1=et, s0=biasp1_ap
                    )

                def psum_mm_groups(win_col, rhs_tile, KT, mp=P):
                    """All NG groups of one m-tile accumulated together so each
                    weight load serves NT consecutive matmuls (same-weight run
                    length 4 is ~70 ns/MM cheaper than 2 on HW). Returns one
                    flat [mp, NB*FD] view per group."""
                    psts = [
                        pspool.tile([P, NB, FD], f32, tag="ps", name="ps")
                        for _ in range(NG)
                    ]
                    for k in range(KT):
                        lhs = win_col(k)
                        for g in range(NG):
                            for n in range(NB):
                                ng = g * NB + n
                                nc.tensor.matmul(
                                    psts[g][:mp, n, :],
                                    lhs,
                                    rhs_tile[:, k, ng * FD : (ng + 1) * FD],
                                    start=(k == 0),
                                    stop=(k == KT - 1),
                                )
                    return [pst[:mp].rearrange("p a b -> p (a b)") for pst in psts]

                def layer(win, bt, btp1, KT, MT, rhs_tile, out_tile):
                    """z = win.T @ rhs + b; out = elu(z)+1 (bf16)."""
                    for m in range(MT):
                        flats = psum_mm_groups(
                            lambda k, m=m: win[:, k, m * P : (m + 1) * P],
                            rhs_tile,
                            KT,
                        )
                        for g in range(NG):
                            elu_wide(
                                flats[g],
                                bt[:, m : m + 1],
                                btp1[:, m : m + 1],
                                out_tile[:, m, g * NB * FD : (g + 1) * NB * FD],
                            )

                def emit_gate_l1():
                    # gate layer 1 (512 -> 256, elu')
                    gp = cpool.tile([P, GH // P, BSH], bf16, tag="gp", name="gp")
                    layer(gw0, gb0t, gb0p1t, OBS // P, GH // P, xt, gp)
                    return gp

                def emit_gate_rest(gp):
                    # gate layer 2 (256 -> 8) + exp
                    expT = cpool.tile([E, BSH], f32, tag="expT", name="expT")
                    gflats = psum_mm_groups(lambda k: gw1[:, k, :], gp, GH // P, mp=E)
                    for g in range(NG):
                        nc.scalar.activation(
                            expT[:, g * NB * FD : (g + 1) * NB * FD],
                            gflats[g],
                            AF.Exp,
                            bias=gb1t[:, 0:1],
                        )
                    # softmax denom: sum over 8 experts via ones-matmul
                    ones = cpool.tile([E, 1], f32, tag="ones", name="ones")
                    nc.vector.memset(ones[:], 1.0)
                    invs = cpool.tile([1, BSH], f32, tag="invs", name="invs")
                    sflats = psum_mm_groups(lambda k: ones[:], expT[:, None, :], 1, mp=1)
                    for g in range(NG):
                        nc.vector.reciprocal(
                            invs[:, g * NB * FD : (g + 1) * NB * FD], sflats[g]
                        )
                    # wT[e, s] = exp(logit_e)/sum (partition-broadcast DMA
                    # needs a DRAM source, so bounce via DRAM)
                    inv_d = dpool.tile([1, BSH], f32, name="inv_d")
                    nc.scalar.dma_start(inv_d[:], invs[:])
                    rep8 = cpool.tile([E, BSH], f32, tag="rep8", name="rep8")
                    nc.scalar.dma_start(
                        rep8[:], inv_d[0:1, :].to_broadcast((E, BSH))
                    )
                    wT = cpool.tile([E, BSH], bf16, tag="wT", name="wT")
                    nc.vector.tensor_tensor(wT[:], expT[:], rep8[:], Alu.mult)
                    wt_d = dpool.tile([E, BSH], bf16, name="wt_d")
                    nc.scalar.dma_start(wt_d[:], wT[:])
                    return wt_d

                def load_expert(e):
                    st = {}
                    st["w0"] = wpool.tile(
                        [P, DIMS[0] // P, DIMS[1]], bf16, tag="w0", name="w0"
                    )
                    nc.sync.dma_start(
                        st["w0"][:], Wd[0][e].rearrange("(ko p) o -> p ko o", p=P)
                    )
                    st["w1"] = wpool.tile(
                        [P, DIMS[1] // P, DIMS[2]], bf16, tag="w1", name="w1"
                    )
                    nc.sync.dma_start(
                        st["w1"][:], Wd[1][e].rearrange("(ko p) o -> p ko o", p=P)
                    )
                    st["w2"] = wpool3.tile(
                        [P, DIMS[2] // P, DIMS[3]], bf16, tag="w2", name="w2"
                    )
                    nc.sync.dma_start(
                        st["w2"][:], Wd[2][e].rearrange("(ko p) o -> p ko o", p=P)
                    )
                    st["w3"] = wpool3.tile(
                        [P, DIMS[3] // P, DIMS[4]], bf16, tag="w3", name="w3"
                    )
                    nc.sync.dma_start(
                        st["w3"][:], Wd[3][e].rearrange("(ko p) o -> p ko o", p=P)
                    )
                    bts = []
                    for l in range(3):
                        bt = bpool.tile([P, MTS[l]], f32, tag=f"b{l}", name=f"b{l}")
                        nc.scalar.dma_start(bt[:], Bd[l][e])
                        btp1 = bpool.tile(
                            [P, MTS[l]], f32, tag=f"b{l}p1", name=f"b{l}p1"
                        )
                        nc.scalar.dma_start(btp1[:], Bp1d[l][e])
                        bts.append((bt, btp1))
                    st["bts"] = bts
                    st["b3"] = bpool.tile([ACT, 1], f32, tag="b3", name="b3")
                    nc.scalar.dma_start(st["b3"][:], B3d[e])
                    return st

                def emit_rw(st, e, wt_d):
                    # gate row for this expert, replicated over 64 partitions
                    st["rw"] = wpool.tile([ACT, BSH], bf16, tag="rw", name="rw")
                    nc.scalar.dma_start(
                        st["rw"][:], wt_d[e : e + 1, :].to_broadcast((ACT, BSH))
                    )

                def emit_L0(st):
                    st["h1"] = hpool.tile(
                        [P, DIMS[1] // P, BSH], bf16, tag="h1", name="h1"
                    )
                    layer(
                        st["w0"], st["bts"][0][0], st["bts"][0][1],
                        DIMS[0] // P, DIMS[1] // P, xt, st["h1"],
                    )

                def emit_L1(st):
                    st["h2"] = hpool.tile(
                        [P, DIMS[2] // P, BSH], bf16, tag="h2", name="h2"
                    )
                    layer(
                        st["w1"], st["bts"][1][0], st["bts"][1][1],
                        DIMS[1] // P, DIMS[2] // P, st["h1"], st["h2"],
                    )

                def emit_L2(st):
                    st["h3"] = hpool.tile(
                        [P, DIMS[3] // P, BSH], bf16, tag="h3", name="h3"
                    )
                    layer(
                        st["w2"], st["bts"][2][0], st["bts"][2][1],
                        DIMS[2] // P, DIMS[3] // P, st["h2"], st["h3"],
                    )

                def emit_L3(st, e, acc):
                    # last layer (256 -> 64), no ELU: acc += w_e * (z + b3)
                    l3flats = psum_mm_groups(
                        lambda k: st["w3"][:, k, :], st["h3"], DIMS[3] // P, mp=ACT
                    )
                    for g in range(NG):
                        gs = slice(g * NB * FD, (g + 1) * NB * FD)
                        ps_flat = l3flats[g]
                        if e == 0:
                            nc.vector.scalar_tensor_tensor(
                                acc[:, gs], ps_flat, st["b3"][:, 0:1], st["rw"][:, gs],
                                Alu.add, Alu.mult,
                            )
                        else:
                            tt = tpool.tile([ACT, NB * FD], f32, tag="t", name="t")
                            nc.vector.scalar_tensor_tensor(
                                tt, ps_flat, st["b3"][:, 0:1], st["rw"][:, gs],
                                Alu.add, Alu.mult,
                            )
                            nc.vector.tensor_add(acc[:, gs], acc[:, gs], tt)

                acc = cpool.tile([ACT, BSH], f32, tag="acc", name="acc")

                if not _PIPE.get("on", True):
                    # serial reference order
                    wt_d = emit_gate_rest(emit_gate_l1())
                    for e in range(E):
                        st = load_expert(e)
                        emit_rw(st, e, wt_d)
                        emit_L0(st)
                        emit_L1(st)
                        emit_L2(st)
                        emit_L3(st, e, acc)
                else:
                    # software pipeline: each layer-boundary ELU drain is
                    # covered by another expert's (or the gate's) matmuls.
                    # tick e: load(e+1), L0(e), [gate|L2(e-1)], rw(e), L1(e),
                    #         L3(e-1)
                    state = {0: load_expert(0)}
                    wt_d = None
                    for e in range(E + 1):
                        if e == 0:
                            gp = emit_gate_l1()
                        if e < E:
                            if e + 1 < E:
                                state[e + 1] = load_expert(e + 1)
                            emit_L0(state[e])
                        if e == 0:
                            wt_d = emit_gate_rest(gp)
                        if e >= 1:
                            emit_L2(state[e - 1])
                        if e < E:
                            emit_rw(state[e], e, wt_d)
                            emit_L1(state[e])
                        if e >= 1:
                            emit_L3(state[e - 1], e - 1, acc)
                            del state[e - 1]

                nc.scalar.dma_start(outd[:], acc[:])

            if reps == 1:
                body()
            else:
                with tc.For_i(0, reps, 1):
                    body()

    nc.compile()
    return nc


def _prep_inputs(inputs):
    """Host-side: shard/transposes/casts + bias folding. Returns in_maps."""
    x = np.asarray(inputs["x"], np.float32)
    Ws = [np.asarray(inputs[f"W{l}"], np.float32) for l in range(4)]
    bs = [np.asarray(inputs[f"b{l}"], np.float32) for l in range(4)]
    gW0 = np.asarray(inputs["gW0"], np.float32)
    gb0 = np.asarray(inputs["gb0"], np.float32)
    gW1 = np.asarray(inputs["gW1"], np.float32)
    gb1 = np.asarray(inputs["gb1"], np.float32)

    shared = {}
    for l in range(4):
        shared[f"W{l}"] = np.ascontiguousarray(Ws[l].astype(BF))
    # effective biases: layer l>0 consumes h' = elu+1, so subtract colsum(W_l)
    beff = [bs[0]] + [bs[l] - Ws[l].sum(axis=1) for l in range(1, 4)]
    MTS = [DIMS[l + 1] // P for l in range(3)]
    for l in range(3):
        pk = beff[l].reshape(E, MTS[l], P).transpose(0, 2, 1)
        shared[f"B{l}"] = np.ascontiguousarray(pk)
        shared[f"B{l}p1"] = np.ascontiguousarray(pk + 1.0)
    shared["B3"] = np.ascontiguousarray(beff[3][:, :, None])
    shared["gW0"] = np.ascontiguousarray(gW0.astype(BF))
    shared["gW1"] = np.ascontiguousarray(gW1.astype(BF))
    gpk = gb0.reshape(GH // P, P).T
    shared["gB0"] = np.ascontiguousarray(gpk)
    shared["gB0p1"] = np.ascontiguousarray(gpk + 1.0)
    shared["gB1"] = np.ascontiguousarray((gb1 - gW1.sum(axis=0))[:, None])

    in_maps = []
    for c in range(NCORES):
        m = dict(shared)
        m["xT"] = np.ascontiguousarray(x[c * BSH : (c + 1) * BSH].T.astype(BF))
        in_maps.append(m)
    return in_maps


def kernel(**inputs):
    from concourse.bass_utils import run_bass_kernel_spmd

    if "nc" not in _cache:
        _cache["nc"] = _build()
    nc = _cache["nc"]
    in_maps = _prep_inputs(inputs)
    res = run_bass_kernel_spmd(nc, in_maps, core_ids=list(range(NCORES)))
    full = np.empty((B, ACT), np.float32)
    for c in range(NCORES):
        full[c * BSH : (c + 1) * BSH] = np.asarray(res.results[c]["out"]).T
    return full
